# revision 3
# baseline (speedup 1.0000x reference)
"""Multi-head attention Trainium2 Bass kernel, v2.

Problem: B=8, S=1024, E=768, H=12, DH=64 MHA with per-head Q/K/V projections
and output projection. Data-parallel over batch: one batch element per
NeuronCore (8 cores).

v2 changes over baseline:
  - x arrives host-transposed (xt [E, S]); no DmaTransposeAnt on device.
  - softmax-denominator reciprocal broadcast via gpsimd partition_broadcast
    (SBUF->SBUF) instead of a DRAM roundtrip DMA pair.
  - output-projection bias added on the PSUM->SBUF copy via DVE
    tensor_tensor with a broadcast bo_rep constant (no K=1 bias matmuls).
  - QKV biases consolidated into one [128, 18] upfront DMA.
  - scores computed as single N=1024 matmuls per (t, head).
  - deferred constant loads (wo) so startup DMAs prioritize x + pair-0 w.

Negative flags in `ablate` to revert: "nopbc", "nobott", "nosc1024",
plus baseline hooks: noattn/nosm/noatmm/nonorm/ttdve/lag1/...
"""
import sys

sys.path.insert(0, "/opt/trn_rl_repo")

import numpy as np
import ml_dtypes
from contextlib import ExitStack, nullcontext

import concourse.bass as bass
import concourse.tile as tile
from concourse import bacc, mybir
from concourse.bass_utils import run_bass_kernel_spmd
from concourse.masks import make_identity

F32 = mybir.dt.float32
BF16 = mybir.dt.bfloat16
AF = mybir.ActivationFunctionType
BF = ml_dtypes.bfloat16

B, S, E, H, DH = 8, 1024, 768, 12, 64
NP_ = 6          # head pairs
ET = 6           # e tiles of 128
ST = 8           # s tiles of 128
NCORES = 8

_cache = {}


def _build_nc(reps=1, ablate=""):
    key = ("nc2", reps, ablate)
    if key in _cache:
        return _cache[key]
    nc = bacc.Bacc("TRN2", target_bir_lowering=False, debug=False,
                   num_devices=NCORES)

    xt = nc.dram_tensor("xt", [E, S], BF16, kind="ExternalInput").ap()
    wq = nc.dram_tensor("wq", [NP_, 128, ET, 128], BF16, kind="ExternalInput").ap()
    wk = nc.dram_tensor("wk", [NP_, 128, ET, 128], BF16, kind="ExternalInput").ap()
    wv = nc.dram_tensor("wv", [NP_, 128, ET, 128], BF16, kind="ExternalInput").ap()
    ball = nc.dram_tensor("ball", [128, NP_ * 3], F32, kind="ExternalInput").ap()
    wo = nc.dram_tensor("wo", [128, ET * E], BF16, kind="ExternalInput").ap()
    bo = nc.dram_tensor("bo", [1, E], BF16, kind="ExternalInput").ap()
    out = nc.dram_tensor("out", [S, E], F32, kind="ExternalOutput").ap()

    pbc = "nopbc" not in ablate
    bott = "nobott" not in ablate
    sc1024 = "sc1024" in ablate  # illegal: matmul may not cross PSUM banks

    with tile.TileContext(nc) as tc, ExitStack() as ctx:
        consts = ctx.enter_context(tc.tile_pool(name="consts", bufs=1))
        xtp = ctx.enter_context(tc.tile_pool(
            name="xtp", bufs=2 if "xtp2" in ablate else 1))
        catp = ctx.enter_context(tc.tile_pool(name="catp", bufs=1))
        wpool = ctx.enter_context(tc.tile_pool(
            name="wpool", bufs=2 if "wp2" in ablate else 3))
        qkp = ctx.enter_context(tc.tile_pool(
            name="qkp", bufs=2 if "qkp2" in ablate else 3))
        vop = ctx.enter_context(tc.tile_pool(
            name="vop", bufs=3 if "lag2" in ablate else 2))
        exq = ctx.enter_context(tc.tile_pool(
            name="exq", bufs=3 if "lag2" in ablate else 2))
        zp = ctx.enter_context(tc.tile_pool(name="zp", bufs=2))
        cup = ctx.enter_context(tc.tile_pool(
            name="cup", bufs=3 if "lag2" in ablate else 2))
        zdp = (None if pbc else
               ctx.enter_context(tc.tile_pool(name="zdp", bufs=4, space="DRAM")))
        osb = ctx.enter_context(tc.tile_pool(name="osb", bufs=3))
        # PSUM: 2 + 4 + 2 = 8 banks
        mmp = ctx.enter_context(tc.tile_pool(name="mmp", bufs=2, space="PSUM"))
        scp = ctx.enter_context(tc.tile_pool(
            name="scp", bufs=1 if "scp1" in ablate else 2, space="PSUM"))
        atp = ctx.enter_context(tc.tile_pool(
            name="atp", bufs=4 if "att4" in ablate else 2, space="PSUM"))

        ident = consts.tile([128, 128], BF16, tag="ident")
        ones_t = consts.tile([1, 128], BF16, tag="ones")
        bo_t = consts.tile([1, E], BF16, tag="bo")
        wo_t = consts.tile([128, ET * E], BF16, tag="wo")
        ball_t = consts.tile([128, NP_ * 3], F32, tag="ball")
        bo_rep = (consts.tile([128, E], BF16, tag="bo_rep", name="bo_rep")
                  if bott else None)
        consts_loaded = [False]

        def load_consts():
            # Emitted after the first pair's weight DMAs so startup
            # bandwidth goes to x and pair-0 weights first.
            if consts_loaded[0]:
                return
            consts_loaded[0] = True
            make_identity(nc, ident)
            nc.gpsimd.memset(ones_t, 1.0)
            nc.sync.dma_start(bo_t, bo)
            nc.sync.dma_start(wo_t, wo)
            if bott:
                nc.gpsimd.partition_broadcast(bo_rep, bo_t)

        nc.sync.dma_start(ball_t, ball)

        for _rep in range(reps):
            # ---- Phase 0: xT [E, S] straight strip loads ----
            xT = [xtp.tile([128, S], BF16, tag=f"xT{et}", name=f"xT{et}")
                  for et in range(ET)]
            for et in range(ET):
                nc.sync.dma_start(xT[et], xt[et * 128:(et + 1) * 128, :])

            catT = [catp.tile([128, S], BF16, tag=f"catT{j}", name=f"catT{j}")
                    for j in range(NP_)]

            def produce(p):
                wq_t = wpool.tile([128, ET, 128], BF16, tag="wq", name="wq_t")
                nc.sync.dma_start(wq_t, wq[p])
                wk_t = wpool.tile([128, ET, 128], BF16, tag="wk", name="wk_t")
                nc.sync.dma_start(wk_t, wk[p])
                wv_t = wpool.tile([128, ET, 128], BF16, tag="wv", name="wv_t")
                nc.sync.dma_start(wv_t, wv[p])
                if p == 0:
                    load_consts()

                qT = qkp.tile([128, S], BF16, tag="qT", name="qT")
                kT = qkp.tile([128, S], BF16, tag="kT", name="kT")
                vT = qkp.tile([128, S], BF16, tag="vT", name="vT")
                for i, (w_t, dst) in enumerate(((wq_t, qT), (wk_t, kT),
                                                (wv_t, vT))):
                    b_ap = ball_t[:, 3 * p + i:3 * p + i + 1]
                    for ch in range(2):
                        pp = mmp.tile([128, 512], F32, tag="mm", name="pp")
                        for et in range(ET):
                            nc.tensor.matmul(
                                pp, w_t[:, et, :],
                                xT[et][:, ch * 512:(ch + 1) * 512],
                                start=(et == 0), stop=(et == ET - 1),
                            )
                        nc.vector.tensor_scalar_add(
                            dst[:, ch * 512:(ch + 1) * 512], pp, b_ap)

                # transpose vT -> v [t, d-pair] packed into vOnes w/ ones cols
                vo = vop.tile([128, ST, 130], BF16, tag="vo", name="vo")
                if "novones" in ablate:
                    nc.gpsimd.memset(vo.rearrange("p t d -> p (t d)"), 1.0)
                else:
                    # only the 2 ones-columns per t-group need init; the d
                    # columns are fully written by the transpose copies
                    nc.vector.memset(
                        vo.rearrange("p t (two dd) -> p t two dd",
                                     two=2)[:, :, :, 64:65], 1.0)
                for sg in range(2):
                    tp = mmp.tile([128, 512], F32, tag="mm",
                                  name="tp").bitcast(BF16)
                    for k in range(4):
                        t = sg * 4 + k
                        nc.tensor.matmul(
                            tp[:, k * 128:(k + 1) * 128],
                            vT[:, t * 128:(t + 1) * 128],
                            ident, is_transpose=True, skip_group_check=True,
                            start=True, stop=True,
                        )
                    dst = vo[:, sg * 4:(sg + 1) * 4, :].rearrange(
                        "p t (two dd) -> p t two dd", two=2)[:, :, :, 0:64]
                    src = tp[:, 0:512].rearrange(
                        "p (t two d) -> p t two d", t=4, two=2)
                    nc.vector.tensor_copy(dst, src)

                if "noattn" in ablate:
                    for e in range(2):
                        nc.vector.memset(catT[p][64 * e:64 * e + 64, :], 0.5)
                    return None
                # scores for both heads interleaved (K=64 row-packing)
                ex_ts = [exq.tile([128, ST, S], BF16, tag=f"ex{e}",
                                  name=f"ex{e}") for e in range(2)]
                for t in range(ST):
                    scs = []
                    for e in range(2):
                        r0 = 64 * e
                        sc = scp.tile([128, S], F32, tag="sc", name="sc")
                        scs.append(sc)
                        if sc1024:
                            nc.tensor.matmul(
                                sc,
                                kT[r0:r0 + 64, t * 128:(t + 1) * 128],
                                qT[r0:r0 + 64, :],
                                tile_position=(r0, 0),
                                start=True, stop=True,
                                skip_group_check=True,
                            )
                        else:
                            for ch in range(2):
                                nc.tensor.matmul(
                                    sc[:, ch * 512:(ch + 1) * 512],
                                    kT[r0:r0 + 64, t * 128:(t + 1) * 128],
                                    qT[r0:r0 + 64, ch * 512:(ch + 1) * 512],
                                    tile_position=(r0, 0),
                                    start=True, stop=True,
                                    skip_group_check=True,
                                )
                    for e in range(2):
                        if "nosm" in ablate:
                            continue
                        nc.scalar.activation(ex_ts[e][:, t, :], scs[e],
                                             AF.Exp, scale=0.125)
                if ("nosm" in ablate) or ("noatmm" in ablate):
                    for e in range(2):
                        nc.vector.memset(catT[p][64 * e:64 * e + 64, :], 0.5)
                    return None
                return (vo, ex_ts)

            def consume(p, state):
                if state is None:
                    return
                vo, ex_ts = state
                catU = cup.tile([128, S], BF16, tag="catU", name="catU")
                deferred = []
                for e in range(2):
                    r0 = 64 * e
                    ex_t = ex_ts[e]
                    for ch in range(2):
                        ap_ = atp.tile([65, 512], F32, tag="att", name="att")
                        for t in range(ST):
                            nc.tensor.matmul(
                                ap_, vo[:, t, 65 * e:65 * e + 65],
                                ex_t[:, t, ch * 512:(ch + 1) * 512],
                                start=(t == 0), stop=(t == ST - 1),
                            )
                        zrec = zp.tile([1, 512], F32, tag="zrec", name="zrec",
                                       bufs=4)
                        prio = (nullcontext() if "lopri" in ablate
                                else tc.high_priority(offset=150))
                        with prio:
                            nc.vector.reciprocal(zrec, ap_[64:65, :])
                            cu_eng = (nc.scalar.copy if "cuact" in ablate
                                      else nc.vector.tensor_copy)
                            cu_eng(
                                catU[r0:r0 + 64, ch * 512:(ch + 1) * 512],
                                ap_[0:64, :])
                        if "nonorm" not in ablate:
                            zrep = zp.tile([128, 512], F32, tag="zrep",
                                           name="zrep", bufs=4)
                            if pbc:
                                nc.gpsimd.partition_broadcast(zrep, zrec)
                            else:
                                zd = zdp.tile([1, 512], F32, tag="zd",
                                              name="zd")
                                nc.sync.dma_start(zd, zrec)
                                nc.sync.dma_start(
                                    zrep, zd.partition_broadcast(128))
                            deferred.append((r0, ch, zrep))
                tt_eng = nc.gpsimd if "ttgps" in ablate else nc.vector
                for r0, ch, zrep in deferred:
                    tt_eng.tensor_tensor(
                        out=catT[p][r0:r0 + 64, ch * 512:(ch + 1) * 512],
                        in0=catU[r0:r0 + 64, ch * 512:(ch + 1) * 512],
                        in1=zrep[r0:r0 + 64, :],
                        op=mybir.AluOpType.mult,
                    )
                if "nonorm" in ablate:
                    nc.vector.tensor_copy(catT[p], catU)

            if "lag2" in ablate:
                states = {}
                for p in range(NP_):
                    states[p] = produce(p)
                    if p >= 2:
                        consume(p - 2, states.pop(p - 2))
                consume(NP_ - 2, states.pop(NP_ - 2))
                consume(NP_ - 1, states.pop(NP_ - 1))
            else:
                prev = None
                for p in range(NP_):
                    state = produce(p)
                    if p >= 1:
                        consume(p - 1, prev)
                    prev = state
                consume(NP_ - 1, prev)

            # ---- Output projection ----
            for st in range(ST):
                for ch in range(2):
                    op_ = mmp.tile([128, 384], F32, tag="mm", name="op")
                    for j in range(NP_):
                        nc.tensor.matmul(
                            op_, catT[j][:, st * 128:(st + 1) * 128],
                            wo_t[:, j * E + ch * 384:j * E + ch * 384 + 384],
                            start=(j == 0),
                            stop=(j == NP_ - 1) if bott else False,
                        )
                    o_sb = osb.tile([128, 384], F32, tag="ot", name="ot")
                    if bott:
                        nc.vector.tensor_tensor(
                            out=o_sb, in0=op_,
                            in1=bo_rep[:, ch * 384:(ch + 1) * 384],
                            op=mybir.AluOpType.add,
                        )
                    else:
                        nc.tensor.matmul(
                            op_, ones_t, bo_t[:, ch * 384:ch * 384 + 384],
                            start=False, stop=True,
                        )
                        nc.vector.tensor_copy(o_sb, op_)
                    nc.sync.dma_start(
                        out[st * 128:(st + 1) * 128,
                            ch * 384:ch * 384 + 384], o_sb)

    nc.compile()
    _cache[key] = nc
    return nc


def _prep_weights(Wq, bq, Wk, bk, Wv, bv, Wo, bo):
    def pack_w(W):  # [12, 768, 64] -> [6, 128, 6, 128] bf16
        Wp = W.reshape(NP_, 2, E, DH).transpose(0, 2, 1, 3).reshape(NP_, E, 128)
        return np.ascontiguousarray(
            Wp.reshape(NP_, ET, 128, 128).transpose(0, 2, 1, 3)).astype(BF)

    def pack_b(b):  # [12, 64] -> [6, 128] f32
        return b.reshape(NP_, 128).astype(np.float32)

    bqp, bkp, bvp = pack_b(bq), pack_b(bk), pack_b(bv)
    ball = np.empty((128, NP_ * 3), np.float32)
    for p in range(NP_):
        ball[:, 3 * p + 0] = bqp[p]
        ball[:, 3 * p + 1] = bkp[p]
        ball[:, 3 * p + 2] = bvp[p]

    return {
        "wq": pack_w(Wq), "wk": pack_w(Wk), "wv": pack_w(Wv),
        "ball": np.ascontiguousarray(ball),
        "wo": np.ascontiguousarray(
            Wo.reshape(ET, 128, E).transpose(1, 0, 2).reshape(128, ET * E)
        ).astype(BF),
        "bo": np.ascontiguousarray(bo.reshape(1, E)).astype(BF),
    }


def kernel(hidden_state, Wq, bq, Wk, bk, Wv, bv, Wo, bo):
    hidden_state = np.asarray(hidden_state, dtype=np.float32)
    shared = _prep_weights(
        np.asarray(Wq, np.float32), np.asarray(bq, np.float32),
        np.asarray(Wk, np.float32), np.asarray(bk, np.float32),
        np.asarray(Wv, np.float32), np.asarray(bv, np.float32),
        np.asarray(Wo, np.float32), np.asarray(bo, np.float32))
    nc = _build_nc()
    in_maps = [
        {"xt": np.ascontiguousarray(hidden_state[b].T).astype(BF), **shared}
        for b in range(NCORES)
    ]
    res = run_bass_kernel_spmd(nc, in_maps, core_ids=list(range(NCORES)))
    return np.stack([r["out"] for r in res.results], axis=0)
